# revision 25
# baseline (speedup 1.0000x reference)
"""Trainium2 Bass kernel for nn_LocalFeatureExtractor (gnn_message_passing).

Math: with per-node features x[b,n,:] (C=128) and K=10 gathered neighbors,
    out = x @ W1^T + W1_b + (conv(feats) + Wc_b) @ W2^T + W2_b
collapses algebraically (fold the two dense layers around the conv) to
    out[b,n] = x[b,n] @ A + sum_k x[b, adj[b,n,k]] @ M_k + bias
with A = W1^T + (W2 Wc_0)^T, M_k = (W2 Wc_k)^T, bias = W1_b + W2_b + W2 Wc_b.

Sharding: data-parallel over batch B=8 -> one graph per NeuronCore.

v2: neighbor gather runs on the DMA engines via SWDGE dma_gather
(transpose mode, bf16): x stays node-major [N, C] in HBM; each gathered
256B row is transpose-written across the 128 SBUF partitions, landing
directly in matmul-rhs orientation [C, n_idx]. GPSIMD only generates
descriptors (~1.3us/call); the 16 DMA engines move the data. This
replaces v1's ap_gather, whose Q7 implementation re-streams the whole
20000-node input per call (~28us/call x 200 calls ~= 5.5ms).

Hardware-probed limits (exceeding these wedges the exec unit):
  - transpose dma_gather: num_idxs <= 1920 per call, single_packet=False
    (single_packet=True caps at ~896).
  - SWDGE ring carveout: num_idxs/16+2 descriptor slots per call out of
    1024 per queue, reclaimed as transfers complete.
  - each SWDGE queue runs desc-gen -> transfer serially; round-robin over
    all 4 queues overlaps them (measured 8.1 -> 1.6 ns/idx from 1 -> 4
    queues on a single core).
"""

import numpy as np
import ml_dtypes

import concourse.bass as bass
import concourse.mybir as mybir
from concourse import bacc
from concourse.tile import TileContext
from concourse.bass_utils import run_bass_kernel_spmd

B, N, C, K = 8, 20000, 128, 10
N_CORES = 8
BLK = 1920                  # nodes per gather call (HW limit: <=1920 idxs)
NFULL = N // BLK            # 10 full blocks
REM = N - NFULL * BLK       # 800-node remainder block
REMPAD = -(-REM // 128) * 128   # 896
SLOTS_F = BLK // 16         # 120 idx slots per full call
SLOTS_R = REMPAD // 16      # 56 idx slots for the remainder call
TOT_SLOTS = K * (NFULL * SLOTS_F + SLOTS_R)   # per-core idx columns
MSTRIP = 480                # matmul strip (cols per PSUM tile; 4 per block)
NQ = 1                      # SWDGE queues. >1 is unsafe on this ucode build:
                            # queues 2-3 return garbage outright, and queues
                            # 0+1 running concurrently corrupt in-flight
                            # transfers when the descriptor FIFO reclaims
                            # (verified by probe; single queue is clean).

_dt = mybir.dt
BF16 = ml_dtypes.bfloat16


def _blocks(n=N):
    """(node_offset, nodes_in_block, padded_idx_count) per gather block."""
    out = []
    off = 0
    while off < n:
        nb = min(BLK, n - off)
        out.append((off, nb, -(-nb // 128) * 128))
        off += nb
    return out


def build(n_cores=N_CORES, reps=1, n=N, gbufs=6, psbufs=2, obufs=3,
          blk_limit=None):
    """Build + compile the per-core Bass program (SPMD: same program, 8 cores)."""
    blocks = _blocks(n)
    if blk_limit is not None:
        blocks = blocks[:blk_limit]
    tot_slots = K * sum(gp // 16 for _, _, gp in blocks)

    nc = bacc.Bacc("TRN2", target_bir_lowering=False, debug=False,
                   num_devices=n_cores, num_swdge_queues=NQ)
    x_rows = nc.dram_tensor("x_rows", [n, C], _dt.bfloat16, kind="ExternalInput").ap()
    xT = nc.dram_tensor("xT", [C, n], _dt.bfloat16, kind="ExternalInput").ap()
    idx = nc.dram_tensor("idx", [C, tot_slots], _dt.int16, kind="ExternalInput").ap()
    wts = nc.dram_tensor("wts", [C, (K + 1) * C], _dt.bfloat16, kind="ExternalInput").ap()
    bias = nc.dram_tensor("bias", [C, 1], _dt.float32, kind="ExternalInput").ap()
    outT = nc.dram_tensor("outT", [C, n], _dt.float32, kind="ExternalOutput").ap()

    with TileContext(nc) as tc:
        with tc.tile_pool(name="const", bufs=1) as cpool, \
             tc.tile_pool(name="gath", bufs=gbufs) as gpool, \
             tc.tile_pool(name="psum", bufs=psbufs, space="PSUM") as ppool, \
             tc.tile_pool(name="outp", bufs=obufs) as opool:
            xT_t = cpool.tile([C, n], _dt.bfloat16)
            idx_t = cpool.tile([C, tot_slots], _dt.int16)
            wts_t = cpool.tile([C, (K + 1) * C], _dt.bfloat16)
            bias_t = cpool.tile([C, 1], _dt.float32)
            nc.sync.dma_start(out=xT_t[:], in_=xT[:])
            nc.sync.dma_start(out=idx_t[:], in_=idx[:])
            nc.sync.dma_start(out=wts_t[:], in_=wts[:])
            nc.sync.dma_start(out=bias_t[:], in_=bias[:])

            for _rep in range(reps):
                scol = 0
                nblk = 0
                for (off, nb, gp) in blocks:
                    slots = gp // 16
                    nstrip = -(-nb // MSTRIP)
                    # k-outer accumulation into nstrip parallel PSUM chains:
                    # each gather buffer is consumed by its strip matmuls as
                    # soon as it lands, bounding SWDGE in-flight depth.
                    pss = []
                    for j in range(nstrip):
                        ps_j = ppool.tile([C, MSTRIP], _dt.float32,
                                          tag="ps%d" % j, name="ps%d" % j)
                        pss.append(ps_j)
                    for k in range(K + 1):
                        if k == 0:
                            g = None
                        else:
                            g = gpool.tile([C, 1, BLK], _dt.bfloat16, tag="g")
                            nc.gpsimd.dma_gather(
                                g[:, :, 0:gp], x_rows[:],
                                idx_t[:, scol:scol + slots],
                                gp, gp, C, transpose=True,
                                single_packet=False,
                                queue_num=nblk % NQ,
                            )
                            scol += slots
                        for j in range(nstrip):
                            c0 = j * MSTRIP
                            cw = min(MSTRIP, nb - c0)
                            rhs = (xT_t[:, off + c0:off + c0 + cw] if k == 0
                                   else g[:, 0, c0:c0 + cw])
                            nc.tensor.matmul(
                                out=pss[j][:, 0:cw],
                                lhsT=wts_t[:, k * C:(k + 1) * C],
                                rhs=rhs,
                                start=(k == 0), stop=(k == K),
                            )
                    nblk += 1
                    for j in range(nstrip):
                        c0 = j * MSTRIP
                        cw = min(MSTRIP, nb - c0)
                        o = opool.tile([C, MSTRIP], _dt.float32)
                        nc.scalar.activation(
                            o[:, 0:cw], pss[j][:, 0:cw],
                            mybir.ActivationFunctionType.Identity,
                            bias=bias_t[:], scale=1.0,
                        )
                        nc.sync.dma_start(
                            out=outT[:, off + c0:off + c0 + cw],
                            in_=o[:, 0:cw])
    nc.compile()
    return nc


def fold_weights(W1_w, W1_b, Wc_w, Wc_b, W2_w, W2_b):
    """Collapse Linear->Conv1d->Linear into 11 [C,C] mats + one bias."""
    W2 = W2_w.astype(np.float64)
    M = np.einsum('de,eck->cdk', W2, Wc_w.astype(np.float64))
    M[:, :, 0] += W1_w.T.astype(np.float64)
    wts = np.concatenate([M[:, :, k] for k in range(K + 1)], axis=1)
    bias = W1_b.astype(np.float64) + W2_b.astype(np.float64) + W2 @ Wc_b.astype(np.float64)
    return wts.astype(np.float32), bias.astype(np.float32).reshape(C, 1)


def make_idx(adj_b, n=N):
    """adj[b] [n,K] -> wrapped int16 gather-index stream [128, TOT_SLOTS].

    One dma_gather call per (node-block, k): call column i = node off+i of
    the block; index j of a call comes from partition j%16, slot j//16
    (replicated across the 8 GPSIMD core groups). Remainder call padded
    with index 0.
    """
    a = np.asarray(adj_b).astype(np.int16)
    cols = []
    for (off, nb, gp) in _blocks(n):
        for k in range(K):
            j = np.zeros(gp, dtype=np.int16)
            j[:nb] = a[off:off + nb, k]
            cols.append(j.reshape(gp // 16, 16).T)     # [16, slots]
    blk = np.concatenate(cols, axis=1)                 # [16, TOT_SLOTS]
    return np.tile(blk, (8, 1)).copy()                 # replicate 8x


def prep_core_inputs(x, adj_mat, wts, bias):
    """Per-core (per-graph) input maps for the SPMD launch."""
    maps = []
    for b in range(B):
        xb = np.asarray(x[b], dtype=np.float32)
        maps.append({
            "x_rows": np.ascontiguousarray(xb).astype(BF16),
            "xT": np.ascontiguousarray(xb.T).astype(BF16),
            "idx": make_idx(adj_mat[b]),
            "wts": wts.astype(BF16),
            "bias": bias,
        })
    return maps


_NC_CACHE = {}


def kernel(x, adj_mat, W1_w, W1_b, Wc_w, Wc_b, W2_w, W2_b):
    x = np.asarray(x)
    adj_mat = np.asarray(adj_mat)
    wts, bias = fold_weights(np.asarray(W1_w), np.asarray(W1_b), np.asarray(Wc_w),
                             np.asarray(Wc_b), np.asarray(W2_w), np.asarray(W2_b))
    if "nc" not in _NC_CACHE:
        _NC_CACHE["nc"] = build()
    nc = _NC_CACHE["nc"]
    in_maps = prep_core_inputs(x, adj_mat, wts, bias)
    res = run_bass_kernel_spmd(nc, in_maps, list(range(N_CORES)))
    out = np.empty((B, N, C), dtype=np.float32)
    for b in range(B):
        out[b] = res.results[b]["outT"].T
    return out
